# revision 3
# baseline (speedup 1.0000x reference)
"""GCN layer (nn_GCNLayer) on 8 Trainium2 NeuronCores via Bass/Tile — v7.

Math:  out = relu(D^-1/2 (A + I) D^-1/2 (x @ W.T))

v4 = pre-gathered edge stream (kernel_v2) + SBUF-resident fp8 one-hot:

  The scatter one-hots S_chunk[e, r] = (rl(e)==r) are built ONCE at
  startup (per-chunk DVE tensor_scalar is_equal, ~880 one-time ops) into
  a persistent fp8 SBUF tensor (~110 KB/partition).  fp8 holds 0/1
  exactly, and the tensor engine accepts mixed bf16 (xg) x fp8 (S)
  matmuls, so iterations touch no S bytes at all:

    per iteration per core: stream xg (~28 MB) sequentially from HBM,
    ~930 matmuls (segment-sum via resident one-hot + projection by W.T
    after aggregation), ~100 activations, 3 MB out.  No gathers, no
    gpsimd, no per-iteration DVE.

  The steady-state `repeat` runs inside a tc.For_i hardware loop so
  program size is independent of repeat.

  v5 scheduling: PSUM->SBUF copies on DVE (frees the scalar engine for
  the relus), output DMAs on the scalar HWDGE ring (loads keep the sync
  ring), PE branch-prefetch hint on the loop back-edge.

  v7 output path: the device writes a transposed bf16 layout
  out[p, t*d + f] = out_row(t*128+p, f) - one contiguous [128, G*d]
  store per tile group (no 512 B scatter descriptors, half the bytes);
  the host transposes back and upcasts to f32.
"""

import sys
import time
from dataclasses import dataclass

import numpy as np
import ml_dtypes

for _p in ("/opt/trn_rl_repo",):
    if _p not in sys.path:
        sys.path.insert(0, _p)

from concourse import bacc, bass, mybir
import concourse.tile as tile
from concourse import bass_utils

P = 128


@dataclass
class Cfg:
    n_nodes: int = 50000
    d: int = 128
    n_cores: int = 8
    dt: str = "bf16"
    tiles_per_group: int = 4

    @property
    def rpc(self):  # rows per core
        return self.n_nodes // self.n_cores

    @property
    def n_tiles(self):
        return (self.rpc + P - 1) // P

    @property
    def np_dt(self):
        return ml_dtypes.bfloat16 if self.dt == "bf16" else np.float32

    @property
    def bir_dt(self):
        return mybir.dt.bfloat16 if self.dt == "bf16" else mybir.dt.float32


# ----------------------------------------------------------------------------
# host-side preprocessing
# ----------------------------------------------------------------------------


def preprocess(cfg: Cfg, x, W, edge_index):
    N, d, C = cfg.n_nodes, cfg.d, cfg.n_cores
    rpc, n_tiles = cfg.rpc, cfg.n_tiles

    x = np.asarray(x, dtype=np.float32)
    W = np.asarray(W, dtype=np.float32)
    row = np.asarray(edge_index[0], dtype=np.int64)
    col = np.asarray(edge_index[1], dtype=np.int64)

    deg = np.bincount(col, minlength=N).astype(np.float64) + 1.0
    dinv = (1.0 / np.sqrt(deg)).astype(np.float32)

    loops = np.arange(N, dtype=np.int64)
    row_a = np.concatenate([row, loops])
    col_a = np.concatenate([col, loops])
    w_a = dinv[row_a] * dinv[col_a]  # [E+N] f32
    owner = row_a // rpc

    rl_all = row_a - owner * rpc
    t_all = rl_all // P
    counts = np.zeros((C, n_tiles), dtype=np.int64)
    np.add.at(counts, (owner, t_all), 1)
    nch = np.maximum(1, -(-counts.max(axis=0) // P))  # [n_tiles]
    cb = np.concatenate([[0], np.cumsum(nch)[:-1]])
    total_chunks = int(nch.sum())

    tpg = cfg.tiles_per_group
    groups = [list(range(s, min(s + tpg, n_tiles))) for s in range(0, n_tiles, tpg)]

    meta = dict(nch=nch, cb=cb, groups=groups, total_chunks=total_chunks)

    WT = np.ascontiguousarray(W.T).astype(cfg.np_dt)  # [in, out]

    per_core = []
    slots = total_chunks * P
    for c in range(C):
        m = owner == c
        rl_c = rl_all[m]
        t_c = t_all[m]
        col_c = col_a[m]
        w_c = w_a[m]
        order = np.argsort(t_c, kind="stable")
        rl_c, t_c, col_c, w_c = rl_c[order], t_c[order], col_c[order], w_c[order]

        cnt = counts[c]
        seg_start = cb * P
        tile_first = np.concatenate([[0], np.cumsum(cnt)[:-1]])
        pos_in_tile = np.arange(len(t_c)) - tile_first[t_c]
        slot = seg_start[t_c] + pos_in_tile

        xg_mat = np.zeros((slots, d), dtype=cfg.np_dt)
        xg_mat[slot] = (x[col_c] * w_c[:, None]).astype(cfg.np_dt)
        rl_vec = np.full(slots, -1.0, dtype=np.float32)
        rl_vec[slot] = (rl_c % P).astype(np.float32)

        # device layout: [128 lanes, chunk-major]: slot s = k*128 + p
        xg_dram = np.ascontiguousarray(
            xg_mat.reshape(total_chunks, P, d).transpose(1, 0, 2).reshape(P, -1)
        )
        rl_dram = np.ascontiguousarray(
            rl_vec.reshape(total_chunks, P).T
        )  # [128, total_chunks] f32
        per_core.append(dict(xg=xg_dram, rl=rl_dram))

    shared = dict(WT=WT)
    return meta, shared, per_core


# ----------------------------------------------------------------------------
# device program
# ----------------------------------------------------------------------------


def build(cfg: Cfg, meta, repeat: int = 1) -> bass.Bass:
    nch = meta["nch"]
    cb = meta["cb"]
    groups = meta["groups"]
    total_chunks = meta["total_chunks"]

    d = cfg.d
    DT = cfg.bir_dt
    F32 = mybir.dt.float32
    FP8 = mybir.dt.float8e4
    n_tiles, rpc = cfg.n_tiles, cfg.rpc

    nc = bacc.Bacc(
        "TRN2",
        target_bir_lowering=False,
        debug=False,
        enable_asserts=False,
        num_devices=cfg.n_cores,
    )

    xg = nc.dram_tensor("xg", [P, total_chunks * d], DT, kind="ExternalInput")
    rl = nc.dram_tensor("rl", [P, total_chunks], F32, kind="ExternalInput")
    WT = nc.dram_tensor("WT", [d, d], DT, kind="ExternalInput")
    out = nc.dram_tensor("out", [P, n_tiles * d], DT, kind="ExternalOutput")

    Relu = mybir.ActivationFunctionType.Relu
    Copy = mybir.ActivationFunctionType.Copy
    eq = mybir.AluOpType.is_equal

    with tile.TileContext(nc) as tc:
        with (
            tc.tile_pool(name="const", bufs=1) as const,
            tc.tile_pool(name="xgp", bufs=3) as xgp,
            tc.tile_pool(name="zp", bufs=4) as zp,
            tc.tile_pool(name="psZ", bufs=4, space="PSUM") as psZ,
            tc.tile_pool(name="psO", bufs=4, space="PSUM") as psO,
            tc.tile_pool(name="op", bufs=3) as op,
        ):
            wt_s = const.tile([d, d], DT)
            nc.sync.dma_start(wt_s[:], WT[:, :])
            rl_s = const.tile([P, total_chunks], F32)
            nc.sync.dma_start(rl_s[:], rl[:, :])
            iota128 = const.tile([P, P], DT)
            nc.gpsimd.iota(
                iota128[:],
                pattern=[[1, P]],
                base=0,
                channel_multiplier=0,
                allow_small_or_imprecise_dtypes=True,
            )
            # one-time: resident fp8 one-hot table S8[p, k*128 + r]
            S8 = const.tile([P, total_chunks * P], FP8)
            for k in range(total_chunks):
                nc.vector.tensor_scalar(
                    S8[:, k * P : (k + 1) * P],
                    iota128[:],
                    rl_s[:, k : k + 1],
                    None,
                    eq,
                )

            with tc.For_i(0, repeat, 1, hint_engines=(mybir.EngineType.PE,)):
                for grp in groups:
                    c0 = int(cb[grp[0]])
                    c1 = int(cb[grp[-1]] + nch[grp[-1]])
                    L = c1 - c0
                    xs = xgp.tile([P, L * d], DT)
                    nc.sync.dma_start(xs[:], xg[:, c0 * d : c1 * d])

                    og = op.tile([P, len(grp) * d], DT)
                    for ti, t in enumerate(grp):
                        K = int(nch[t])
                        base = int(cb[t]) - c0
                        ps_z = psZ.tile([P, d], F32)
                        for j in range(K):
                            nc.tensor.matmul(
                                ps_z[:],
                                xs[:, (base + j) * d : (base + j + 1) * d],
                                S8[:, (cb[t] + j) * P : (cb[t] + j + 1) * P],
                                start=(j == 0),
                                stop=(j == K - 1),
                            )
                        zT = zp.tile([P, d], DT)
                        nc.vector.tensor_copy(zT[:], ps_z[:])
                        ps_o = psO.tile([P, d], F32)
                        nc.tensor.matmul(
                            ps_o[:], zT[:], wt_s[:], start=True, stop=True
                        )
                        nc.scalar.activation(
                            og[:, ti * d : (ti + 1) * d], ps_o[:], Relu
                        )

                    t0 = grp[0]
                    nc.scalar.dma_start(
                        out[:, t0 * d : (t0 + len(grp)) * d], og[:]
                    )

    nc.compile()
    return nc


# ----------------------------------------------------------------------------
# entry point
# ----------------------------------------------------------------------------

_last_results = None


def kernel(x, W, edge_index):
    cfg = Cfg()
    meta, shared, per_core = preprocess(cfg, x, W, edge_index)
    nc = build(cfg, meta)

    in_maps = [
        {"xg": pc["xg"], "rl": pc["rl"], "WT": shared["WT"]} for pc in per_core
    ]
    res = None
    for attempt in range(4):
        try:
            res = bass_utils.run_bass_kernel_spmd(
                nc, in_maps, core_ids=list(range(cfg.n_cores))
            )
            break
        except Exception:
            if attempt == 3:
                raise
            time.sleep(45)
    global _last_results
    _last_results = res
    out = np.concatenate([unpack_out(cfg, r["out"]) for r in res.results], axis=0)
    return out


def unpack_out(cfg, out_dev):
    """[128, n_tiles*d] bf16 device layout -> [rpc, d] f32 rows."""
    full = (
        np.asarray(out_dev)
        .reshape(P, cfg.n_tiles, cfg.d)
        .transpose(1, 0, 2)
        .reshape(cfg.n_tiles * P, cfg.d)
        .astype(np.float32)
    )
    return full[: cfg.rpc]
